# revision 4
# baseline (speedup 1.0000x reference)
"""Self-contained Trainium2 Bass kernel for the 4-layer alternating-direction
GRU stack (nn_BiGRU): B=32, T=1024, DIN=H=768, L=4, fp32.

Sharding: data-parallel over batch across 8 NeuronCores (4 sequences/core);
GRU weights replicated (shipped bf16 to cut tunnel I/O).

Layout: everything transposed (feature dim on SBUF partitions). All tensors
are stored in the ORIGINAL time axis tau; the per-layer direction flips of
the reference are realized by alternating the scan direction (even layers
scan tau ascending, odd layers descending) and the residual becomes a plain
tau-aligned add: o_l(tau) = o_{l-1}(tau) + hhat_l(tau). No data reversal or
transposes anywhere.

Per layer: (1) input GEMM xgT = W_ih^T-blocks @ xT in [128,NT]-column chunks
(PSUM f32, bias folded into the PSUM->SBUF Identity-activation copy);
(2) sequential scan with hgT = W_hh-blocks @ hT as 18x6 [128,4] fp32r
matmuls + identity-matmul injection of xg(r,z) and K=1 bias inject for the
n gate, gates on ACT/DVE in [128,6,4] tiles, h kept in hT layout so no
transpose is needed between steps.
"""

import sys
import numpy as np

sys.path.insert(0, "/opt/trn_rl_repo")

import concourse.bacc as bacc
import concourse.bass as bass
import concourse.mybir as mybir
import concourse.tile as tile
from concourse.bass_utils import run_bass_kernel_spmd
from contextlib import ExitStack
from ml_dtypes import bfloat16

F32 = mybir.dt.float32
F32R = mybir.dt.float32r
BF16 = mybir.dt.bfloat16
AF = mybir.ActivationFunctionType

N_CORES = 8
B_FULL, T_FULL, DIN, H, L = 32, 1024, 768, 768, 4
B = B_FULL // N_CORES   # 4 sequences per core
G = 3 * H               # 2304
KC = H // 128           # 6 contraction chunks
GC = G // 128           # 18 gate-row chunks
U = 32                  # scan steps per For_i iteration
NT = 512                # tokens per input-GEMM chunk


def _r(ap):
    return ap.bitcast(F32R)


def build_gru(nc, tc, ctx, T=T_FULL, U=U, NT=NT):
    NCH = (T * B) // NT     # GEMM chunks per layer
    UB = U * B              # columns per scan block
    assert T % U == 0 and U % 2 == 0 and (T * B) % NT == 0

    xT0 = nc.dram_tensor("xT0", [128, KC, T * B], BF16, kind="ExternalInput")
    wih, whh, gbt, hbt = [], [], [], []
    for l in range(L):
        wih.append(nc.dram_tensor(f"wih{l}", [H, G], BF16, kind="ExternalInput"))
        whh.append(nc.dram_tensor(f"whh{l}", [H, G], BF16, kind="ExternalInput"))
        gbt.append(nc.dram_tensor(f"gbt{l}", [128, GC], F32, kind="ExternalInput"))
        hbt.append(nc.dram_tensor(f"hbt{l}", [1, H], F32, kind="ExternalInput"))
    idn = nc.dram_tensor("idn", [128, 128], F32, kind="ExternalInput")
    ones = nc.dram_tensor("ones", [1, 128], F32, kind="ExternalInput")

    xgT = nc.dram_tensor("xgT", [128, GC, T * B], F32)
    oA = nc.dram_tensor("oA", [128, KC, T * B], BF16)
    oB = nc.dram_tensor("oB", [128, KC, T * B], BF16)
    out = nc.dram_tensor("out", [128, KC, T * B], BF16, kind="ExternalOutput")

    cpool = ctx.enter_context(tc.tile_pool(name="const", bufs=1))
    t_id = cpool.tile([128, 128], F32)
    nc.sync.dma_start(out=t_id[:], in_=idn[:])
    t_ones = cpool.tile([1, 128], F32)
    nc.sync.dma_start(out=t_ones[:], in_=ones[:])

    for l in range(L):
        src = [xT0, oA, oB, oA][l]   # GEMM input / residual source
        dst = [oA, oB, oA, out][l]   # residual output o_l
        fwd = (l % 2 == 0)           # scan direction in tau

        # ================= input GEMM phase =================
        with tc.tile_pool(name=f"gw{l}", bufs=1) as wpool, \
             tc.tile_pool(name=f"gx{l}", bufs=2) as xpool, \
             tc.tile_pool(name=f"gs{l}", bufs=4) as spool, \
             tc.tile_pool(name=f"gp{l}", bufs=4, space="PSUM") as ppool:
            t_wih = []
            for k in range(KC):
                w = wpool.tile([128, G], BF16, tag=f"wih{k}", name=f"wih_t{l}_{k}")
                nc.sync.dma_start(out=w[:], in_=wih[l][128 * k:128 * (k + 1), :])
                t_wih.append(w)
            t_gb = wpool.tile([128, GC], F32, tag="gb")
            nc.sync.dma_start(out=t_gb[:], in_=gbt[l][:])

            for c in range(NCH):
                xin = xpool.tile([128, KC, NT], BF16, tag="xin", name=f"xin{l}")
                nc.sync.dma_start(out=xin[:], in_=src[:, :, NT * c:NT * (c + 1)])
                for g in range(GC):
                    ps = ppool.tile([128, NT], F32, tag="ps", name=f"ps{l}")
                    for k in range(KC):
                        nc.tensor.matmul(
                            ps[:], t_wih[k][:, 128 * g:128 * (g + 1)],
                            xin[:, k, :], start=(k == 0), stop=(k == KC - 1))
                    sg = spool.tile([128, NT], F32, tag="sg", name=f"sg{l}")
                    nc.scalar.activation(sg[:], ps[:], AF.Identity,
                                         bias=t_gb[:, g:g + 1])
                    nc.sync.dma_start(
                        out=xgT[:, g, NT * c:NT * (c + 1)], in_=sg[:])

        # ================= scan phase =================
        with tc.tile_pool(name=f"sw{l}", bufs=1) as wpool, \
             tc.tile_pool(name=f"sc{l}", bufs=2) as cvt, \
             tc.tile_pool(name=f"sx{l}", bufs=1) as xpool, \
             tc.tile_pool(name=f"sh{l}", bufs=1) as hpool, \
             tc.tile_pool(name=f"sg{l}", bufs=2) as gpool, \
             tc.tile_pool(name=f"sp{l}", bufs=2, space="PSUM") as ppool:
            t_whh = []
            for k in range(KC):
                wb = cvt.tile([128, G], BF16, tag="wb", name=f"wb{l}")
                nc.sync.dma_start(out=wb[:], in_=whh[l][128 * k:128 * (k + 1), :])
                w = wpool.tile([128, G], F32, tag=f"whh{k}", name=f"whh_t{l}_{k}")
                nc.scalar.activation(w[:], wb[:], AF.Copy)
                t_whh.append(w)
            t_hb = wpool.tile([1, H], F32, tag="hb")
            nc.sync.dma_start(out=t_hb[:], in_=hbt[l][:])

            h = [hpool.tile([128, KC, B], F32, tag=f"h{p}", name=f"h{p}_{l}")
                 for p in range(2)]
            nc.vector.memset(h[0][:], 0.0)

            with tc.For_i(0, T * B, UB) as i:
                c0 = bass.ds(i, UB) if fwd else bass.ds((T - U) * B - i, UB)
                xgs = xpool.tile([128, GC, UB], F32, tag="xgs", name=f"xgs{l}")
                nc.sync.dma_start(out=xgs[:], in_=xgT[:, :, c0])
                if l > 0:
                    pvb = xpool.tile([128, KC, UB], BF16, tag="pvb",
                                     name=f"pvb{l}")
                    nc.sync.dma_start(out=pvb[:], in_=src[:, :, c0])
                    pvf = xpool.tile([128, KC, UB], F32, tag="pvf",
                                     name=f"pvf{l}")
                    nc.gpsimd.tensor_copy(pvf[:], pvb[:])
                ob = xpool.tile([128, KC, UB], BF16, tag="ob", name=f"ob{l}")

                for k in range(U):
                    j = k if fwd else U - 1 - k   # intra-block tau index
                    p, q = k % 2, 1 - k % 2
                    phr = ppool.tile([128, KC, B], F32, tag="phr",
                                     name=f"phr{l}")
                    phz = ppool.tile([128, KC, B], F32, tag="phz",
                                     name=f"phz{l}")
                    phn = ppool.tile([128, KC, B], F32, tag="phn",
                                     name=f"phn{l}")
                    # gate-row chunks in G: r = 0..5, z = 6..11, n = 12..17.
                    # Emission order r, n, z: the r->sigmoid->t1 chain starts
                    # as early as possible, z (fully off-chain) goes last.
                    for gg, ps_t, goff in ((0, phr, 0), (2, phn, 2 * KC),
                                           (1, phz, KC)):
                        for c in range(KC):
                            g = goff + c
                            for k6 in range(KC):
                                nc.tensor.matmul(
                                    ps_t[:, c, :],
                                    _r(t_whh[k6])[:, 128 * g:128 * (g + 1)],
                                    _r(h[p])[:, k6, :],
                                    start=(k6 == 0), stop=False)
                            if gg < 2:
                                nc.tensor.matmul(
                                    ps_t[:, c, :], _r(t_id)[:],
                                    _r(xgs)[:, g, B * j:B * (j + 1)],
                                    start=False, stop=True)
                            else:
                                nc.tensor.matmul(
                                    ps_t[:, c, :],
                                    _r(t_hb)[:, 128 * c:128 * (c + 1)],
                                    _r(t_ones)[:, 0:B],
                                    start=False, stop=True)
                    r_t = ppool.tile([128, KC, B], F32, tag="r", name=f"r{l}")
                    nc.scalar.activation(r_t[:], phr[:], AF.Sigmoid)
                    # z/oz before tanh in the ACT queue: their dep (phz) is
                    # ready well before tanh's (t2), and v waits on oz.
                    z_t = gpool.tile([128, KC, B], F32, tag="z", name=f"z{l}")
                    nc.scalar.activation(z_t[:], phz[:], AF.Sigmoid)
                    oz_t = gpool.tile([128, KC, B], F32, tag="oz",
                                      name=f"oz{l}")
                    nc.scalar.activation(oz_t[:], phz[:], AF.Sigmoid,
                                         scale=-1.0)
                    t1 = gpool.tile([128, KC, B], F32, tag="t1", name=f"t1{l}")
                    nc.vector.tensor_mul(t1[:], r_t[:], phn[:])
                    t2 = gpool.tile([128, KC, B], F32, tag="t2", name=f"t2{l}")
                    nc.vector.tensor_add(t2[:], t1[:],
                                         xgs[:, 2 * KC:GC, B * j:B * (j + 1)])
                    tn = gpool.tile([128, KC, B], F32, tag="tn", name=f"tn{l}")
                    nc.scalar.activation(tn[:], t2[:], AF.Tanh)
                    u_t = gpool.tile([128, KC, B], F32, tag="u", name=f"u{l}")
                    nc.gpsimd.tensor_mul(u_t[:], z_t[:], h[p][:])
                    v_t = gpool.tile([128, KC, B], F32, tag="v", name=f"v{l}")
                    nc.vector.tensor_mul(v_t[:], oz_t[:], tn[:])
                    nc.vector.tensor_add(h[q][:], v_t[:], u_t[:])
                    # residual output o_l = o_{l-1} + hhat_l (bf16)
                    if l == 0:
                        nc.gpsimd.tensor_copy(ob[:, :, B * j:B * (j + 1)],
                                              h[q][:])
                    else:
                        nc.gpsimd.tensor_add(ob[:, :, B * j:B * (j + 1)],
                                             h[q][:],
                                             pvf[:, :, B * j:B * (j + 1)])
                nc.sync.dma_start(out=dst[:, :, c0], in_=ob[:])
    return out


def prep_inputs(inputs, core, n_cores=N_CORES, T=T_FULL):
    x = np.asarray(inputs["x"])[core * B:(core + 1) * B, :T]   # [B, T, DIN]
    xT = np.ascontiguousarray(x.transpose(2, 1, 0).reshape(DIN, T * B))
    m = {
        "xT0": np.ascontiguousarray(
            xT.reshape(KC, 128, T * B).transpose(1, 0, 2)).astype(bfloat16),
        "idn": np.eye(128, dtype=np.float32),
        "ones": np.ones((1, 128), dtype=np.float32),
    }
    for l in range(L):
        if l == 0:
            Wi, Wh = inputs["W_ih0"], inputs["W_hh0"]
            bi, bh = inputs["b_ih0"], inputs["b_hh0"]
        else:
            Wi, Wh = inputs["W_ih_s"][l - 1], inputs["W_hh_s"][l - 1]
            bi, bh = inputs["b_ih_s"][l - 1], inputs["b_hh_s"][l - 1]
        m[f"wih{l}"] = np.ascontiguousarray(np.asarray(Wi).T).astype(bfloat16)
        m[f"whh{l}"] = np.ascontiguousarray(np.asarray(Wh).T).astype(bfloat16)
        gb = np.asarray(bi, dtype=np.float32).copy()
        gb[:2 * H] += np.asarray(bh)[:2 * H]
        m[f"gbt{l}"] = np.ascontiguousarray(gb.reshape(GC, 128).T)
        m[f"hbt{l}"] = np.asarray(bh)[2 * H:].copy().reshape(1, H).astype(
            np.float32)
    return m


def finish_output(results, T=T_FULL):
    outs = []
    for rdict in results:
        o = np.asarray(rdict["out"]).astype(np.float32)   # [128, KC, T*B]
        o = o.transpose(1, 0, 2).reshape(H, T, B).transpose(2, 1, 0)
        outs.append(o)
    return np.ascontiguousarray(np.concatenate(outs, axis=0))


_NC_CACHE = {}


def _get_nc(T=T_FULL):
    if T not in _NC_CACHE:
        nc = bacc.Bacc("TRN2", target_bir_lowering=False, debug=False,
                       num_devices=N_CORES)
        with tile.TileContext(nc) as tc:
            with ExitStack() as ctx:
                build_gru(nc, tc, ctx, T=T)
        nc.compile()
        _NC_CACHE[T] = nc
    return _NC_CACHE[T]


def run(inputs, trace=False, **spmd_kwargs):
    nc = _get_nc()
    in_maps = [prep_inputs(inputs, core) for core in range(N_CORES)]
    res = run_bass_kernel_spmd(nc, in_maps, core_ids=list(range(N_CORES)),
                               trace=trace, **spmd_kwargs)
    return finish_output(res.results), res


def kernel(**inputs):
    out, _ = run(inputs)
    return out


# revision 7
# speedup vs baseline: 11.1769x; 11.1769x over previous
"""Self-contained Trainium2 Bass kernel for the 4-layer alternating-direction
GRU stack (nn_BiGRU): B=32, T=1024, DIN=H=768, L=4, fp32.

Sharding: data-parallel over batch across 8 NeuronCores (4 sequences/core);
GRU weights replicated (shipped bf16 to cut tunnel I/O).

Layout: everything transposed (feature dim on SBUF partitions). All tensors
are stored in the ORIGINAL time axis tau; the per-layer direction flips of
the reference are realized by alternating the scan direction (even layers
scan tau ascending, odd layers descending) and the residual becomes a plain
tau-aligned add: o_l(tau) = o_{l-1}(tau) + hhat_l(tau). No data reversal or
transposes anywhere.

Per layer: (1) input GEMM xgT = W_ih^T-blocks @ xT in [128,NT]-column chunks
(PSUM f32, bias folded into the PSUM->SBUF Identity-activation copy);
(2) sequential scan with hgT = W_hh-blocks @ hT as 18x6 [128,4] fp32r
matmuls + identity-matmul injection of xg(r,z) and K=1 bias inject for the
n gate, gates on ACT/DVE in [128,6,4] tiles, h kept in hT layout so no
transpose is needed between steps.
"""

import sys
import numpy as np

sys.path.insert(0, "/opt/trn_rl_repo")

import concourse.bacc as bacc
import concourse.bass as bass
import concourse.mybir as mybir
import concourse.tile as tile
from concourse.bass_utils import run_bass_kernel_spmd
from contextlib import ExitStack
from ml_dtypes import bfloat16

F32 = mybir.dt.float32
F32R = mybir.dt.float32r
BF16 = mybir.dt.bfloat16
AF = mybir.ActivationFunctionType

N_CORES = 8
B_FULL, T_FULL, DIN, H, L = 32, 1024, 768, 768, 4
B = B_FULL // N_CORES   # 4 sequences per core
G = 3 * H               # 2304
KC = H // 128           # 6 contraction chunks
GC = G // 128           # 18 gate-row chunks
U = 64                  # scan steps per For_i iteration
NT = 512                # tokens per input-GEMM chunk


def _r(ap):
    return ap.bitcast(F32R)


def build_gru(nc, tc, ctx, T=T_FULL, U=U, NT=NT):
    NCH = (T * B) // NT     # GEMM chunks per layer
    UB = U * B              # columns per scan block
    assert T % U == 0 and U % 2 == 0 and (T * B) % NT == 0

    xT0 = nc.dram_tensor("xT0", [128, KC, T * B], BF16, kind="ExternalInput")
    wih, whh, gbt, hbt = [], [], [], []
    for l in range(L):
        wih.append(nc.dram_tensor(f"wih{l}", [H, G], BF16, kind="ExternalInput"))
        whh.append(nc.dram_tensor(f"whh{l}", [H, G], BF16, kind="ExternalInput"))
        gbt.append(nc.dram_tensor(f"gbt{l}", [128, GC], F32, kind="ExternalInput"))
        hbt.append(nc.dram_tensor(f"hbt{l}", [1, H], F32, kind="ExternalInput"))
    idn = nc.dram_tensor("idn", [128, 128], F32, kind="ExternalInput")
    ones = nc.dram_tensor("ones", [1, 128], F32, kind="ExternalInput")

    xgT = nc.dram_tensor("xgT", [128, GC, T * B], F32)
    oA = nc.dram_tensor("oA", [128, KC, T * B], BF16)
    oB = nc.dram_tensor("oB", [128, KC, T * B], BF16)
    out = nc.dram_tensor("out", [128, KC, T * B], BF16, kind="ExternalOutput")

    cpool = ctx.enter_context(tc.tile_pool(name="const", bufs=1))
    t_id = cpool.tile([128, 128], F32)
    nc.sync.dma_start(out=t_id[:], in_=idn[:])
    t_ones = cpool.tile([1, 128], F32)
    nc.sync.dma_start(out=t_ones[:], in_=ones[:])

    for l in range(L):
        src = [xT0, oA, oB, oA][l]   # GEMM input / residual source
        dst = [oA, oB, oA, out][l]   # residual output o_l
        fwd = (l % 2 == 0)           # scan direction in tau

        # ================= input GEMM phase =================
        with tc.tile_pool(name=f"gw{l}", bufs=1) as wpool, \
             tc.tile_pool(name=f"gx{l}", bufs=2) as xpool, \
             tc.tile_pool(name=f"gs{l}", bufs=4) as spool, \
             tc.tile_pool(name=f"gp{l}", bufs=4, space="PSUM") as ppool:
            t_wih = []
            for k in range(KC):
                w = wpool.tile([128, G], BF16, tag=f"wih{k}", name=f"wih_t{l}_{k}")
                nc.sync.dma_start(out=w[:], in_=wih[l][128 * k:128 * (k + 1), :])
                t_wih.append(w)
            t_gb = wpool.tile([128, GC], F32, tag="gb")
            nc.sync.dma_start(out=t_gb[:], in_=gbt[l][:])

            for c in range(NCH):
                xin = xpool.tile([128, KC, NT], BF16, tag="xin", name=f"xin{l}")
                nc.sync.dma_start(out=xin[:], in_=src[:, :, NT * c:NT * (c + 1)])
                for g in range(GC):
                    ps = ppool.tile([128, NT], F32, tag="ps", name=f"ps{l}")
                    for k in range(KC):
                        nc.tensor.matmul(
                            ps[:], t_wih[k][:, 128 * g:128 * (g + 1)],
                            xin[:, k, :], start=(k == 0), stop=(k == KC - 1))
                    sg = spool.tile([128, NT], F32, tag="sg", name=f"sg{l}")
                    nc.scalar.activation(sg[:], ps[:], AF.Identity,
                                         bias=t_gb[:, g:g + 1])
                    nc.sync.dma_start(
                        out=xgT[:, g, NT * c:NT * (c + 1)], in_=sg[:])

        # ================= scan phase =================
        with tc.tile_pool(name=f"sw{l}", bufs=1) as wpool, \
             tc.tile_pool(name=f"sc{l}", bufs=2) as cvt, \
             tc.tile_pool(name=f"sx{l}", bufs=1) as xpool, \
             tc.tile_pool(name=f"sh{l}", bufs=1) as hpool, \
             tc.tile_pool(name=f"sg{l}", bufs=2) as gpool, \
             tc.tile_pool(name=f"sp{l}", bufs=2, space="PSUM") as ppool:
            t_whh = []
            for k in range(KC):
                wb = cvt.tile([128, G], BF16, tag="wb", name=f"wb{l}")
                nc.sync.dma_start(out=wb[:], in_=whh[l][128 * k:128 * (k + 1), :])
                w = wpool.tile([128, G], F32, tag=f"whh{k}", name=f"whh_t{l}_{k}")
                nc.scalar.activation(w[:], wb[:], AF.Copy)
                t_whh.append(w)
            t_hb = wpool.tile([1, H], F32, tag="hb")
            nc.sync.dma_start(out=t_hb[:], in_=hbt[l][:])

            h = [hpool.tile([128, KC, B], F32, tag=f"h{p}", name=f"h{p}_{l}")
                 for p in range(2)]
            nc.vector.memset(h[0][:], 0.0)
            ones_t = hpool.tile([128, KC, B], F32, tag="onest",
                                name=f"onest{l}")
            nc.vector.memset(ones_t[:], 1.0)

            with tc.For_i(0, T * B, UB) as i:
                c0 = bass.ds(i, UB) if fwd else bass.ds((T - U) * B - i, UB)
                xgs = xpool.tile([128, GC, UB], F32, tag="xgs", name=f"xgs{l}")
                # split by gate group: the r-gate injections (first consumers)
                # only wait for the first third of the block data
                nc.sync.dma_start(out=xgs[:, 0:KC, :],
                                  in_=xgT[:, 0:KC, c0])
                nc.sync.dma_start(out=xgs[:, 2 * KC:GC, :],
                                  in_=xgT[:, 2 * KC:GC, c0])
                nc.sync.dma_start(out=xgs[:, KC:2 * KC, :],
                                  in_=xgT[:, KC:2 * KC, c0])
                if l > 0:
                    pvb = xpool.tile([128, KC, UB], BF16, tag="pvb",
                                     name=f"pvb{l}")
                    nc.sync.dma_start(out=pvb[:], in_=src[:, :, c0])
                    pvf = xpool.tile([128, KC, UB], F32, tag="pvf",
                                     name=f"pvf{l}")
                    nc.gpsimd.tensor_copy(pvf[:], pvb[:])
                ob = xpool.tile([128, KC, UB], BF16, tag="ob", name=f"ob{l}")

                for k in range(U):
                    j = k if fwd else U - 1 - k   # intra-block tau index
                    p, q = k % 2, 1 - k % 2
                    phr = ppool.tile([128, KC, B], F32, tag="phr",
                                     name=f"phr{l}")
                    phz = ppool.tile([128, KC, B], F32, tag="phz",
                                     name=f"phz{l}")
                    phn = ppool.tile([128, KC, B], F32, tag="phn",
                                     name=f"phn{l}")
                    # gate-row chunks in G: r = 0..5, z = 6..11, n = 12..17.
                    # Emission order r, n, z: the r->sigmoid->t1 chain starts
                    # as early as possible, z (fully off-chain) goes last.
                    for gg, ps_t, goff in ((0, phr, 0), (2, phn, 2 * KC),
                                           (1, phz, KC)):
                        for c in range(KC):
                            g = goff + c
                            for k6 in range(KC):
                                nc.tensor.matmul(
                                    ps_t[:, c, :],
                                    _r(t_whh[k6])[:, 128 * g:128 * (g + 1)],
                                    _r(h[p])[:, k6, :],
                                    start=(k6 == 0), stop=False)
                            if gg < 2:
                                nc.tensor.matmul(
                                    ps_t[:, c, :], _r(t_id)[:],
                                    _r(xgs)[:, g, B * j:B * (j + 1)],
                                    start=False, stop=True)
                            else:
                                nc.tensor.matmul(
                                    ps_t[:, c, :],
                                    _r(t_hb)[:, 128 * c:128 * (c + 1)],
                                    _r(t_ones)[:, 0:B],
                                    start=False, stop=True)
                    r_t = ppool.tile([128, KC, B], F32, tag="r", name=f"r{l}")
                    nc.scalar.activation(r_t[:], phr[:], AF.Sigmoid)
                    z_t = gpool.tile([128, KC, B], F32, tag="z", name=f"z{l}")
                    nc.scalar.activation(z_t[:], phz[:], AF.Sigmoid)
                    # 1-z on DVE keeps the congested ACT queue at 3 ops/step
                    oz_t = gpool.tile([128, KC, B], F32, tag="oz",
                                      name=f"oz{l}")
                    nc.vector.scalar_tensor_tensor(
                        oz_t[:], z_t[:], -1.0, ones_t[:],
                        mybir.AluOpType.mult, mybir.AluOpType.add)
                    t1 = gpool.tile([128, KC, B], F32, tag="t1", name=f"t1{l}")
                    nc.vector.tensor_mul(t1[:], r_t[:], phn[:])
                    t2 = gpool.tile([128, KC, B], F32, tag="t2", name=f"t2{l}")
                    nc.vector.tensor_add(t2[:], t1[:],
                                         xgs[:, 2 * KC:GC, B * j:B * (j + 1)])
                    tn = gpool.tile([128, KC, B], F32, tag="tn", name=f"tn{l}")
                    nc.scalar.activation(tn[:], t2[:], AF.Tanh)
                    u_t = gpool.tile([128, KC, B], F32, tag="u", name=f"u{l}")
                    nc.gpsimd.tensor_mul(u_t[:], z_t[:], h[p][:])
                    v_t = gpool.tile([128, KC, B], F32, tag="v", name=f"v{l}")
                    nc.vector.tensor_mul(v_t[:], oz_t[:], tn[:])
                    nc.vector.tensor_add(h[q][:], v_t[:], u_t[:])
                    # residual output o_l = o_{l-1} + hhat_l (bf16)
                    if l == 0:
                        nc.gpsimd.tensor_copy(ob[:, :, B * j:B * (j + 1)],
                                              h[q][:])
                    else:
                        nc.gpsimd.tensor_add(ob[:, :, B * j:B * (j + 1)],
                                             h[q][:],
                                             pvf[:, :, B * j:B * (j + 1)])
                nc.sync.dma_start(out=dst[:, :, c0], in_=ob[:])
    return out


def prep_inputs(inputs, core, n_cores=N_CORES, T=T_FULL):
    x = np.asarray(inputs["x"])[core * B:(core + 1) * B, :T]   # [B, T, DIN]
    xT = np.ascontiguousarray(x.transpose(2, 1, 0).reshape(DIN, T * B))
    m = {
        "xT0": np.ascontiguousarray(
            xT.reshape(KC, 128, T * B).transpose(1, 0, 2)).astype(bfloat16),
        "idn": np.eye(128, dtype=np.float32),
        "ones": np.ones((1, 128), dtype=np.float32),
    }
    for l in range(L):
        if l == 0:
            Wi, Wh = inputs["W_ih0"], inputs["W_hh0"]
            bi, bh = inputs["b_ih0"], inputs["b_hh0"]
        else:
            Wi, Wh = inputs["W_ih_s"][l - 1], inputs["W_hh_s"][l - 1]
            bi, bh = inputs["b_ih_s"][l - 1], inputs["b_hh_s"][l - 1]
        m[f"wih{l}"] = np.ascontiguousarray(np.asarray(Wi).T).astype(bfloat16)
        m[f"whh{l}"] = np.ascontiguousarray(np.asarray(Wh).T).astype(bfloat16)
        gb = np.asarray(bi, dtype=np.float32).copy()
        gb[:2 * H] += np.asarray(bh)[:2 * H]
        m[f"gbt{l}"] = np.ascontiguousarray(gb.reshape(GC, 128).T)
        m[f"hbt{l}"] = np.asarray(bh)[2 * H:].copy().reshape(1, H).astype(
            np.float32)
    return m


def finish_output(results, T=T_FULL):
    outs = []
    for rdict in results:
        o = np.asarray(rdict["out"]).astype(np.float32)   # [128, KC, T*B]
        o = o.transpose(1, 0, 2).reshape(H, T, B).transpose(2, 1, 0)
        outs.append(o)
    return np.ascontiguousarray(np.concatenate(outs, axis=0))


_NC_CACHE = {}


def _get_nc(T=T_FULL):
    if T not in _NC_CACHE:
        nc = bacc.Bacc("TRN2", target_bir_lowering=False, debug=False,
                       num_devices=N_CORES)
        with tile.TileContext(nc) as tc:
            with ExitStack() as ctx:
                build_gru(nc, tc, ctx, T=T)
        nc.compile()
        _NC_CACHE[T] = nc
    return _NC_CACHE[T]


def run(inputs, trace=False, **spmd_kwargs):
    nc = _get_nc()
    in_maps = [prep_inputs(inputs, core) for core in range(N_CORES)]
    res = run_bass_kernel_spmd(nc, in_maps, core_ids=list(range(N_CORES)),
                               trace=trace, **spmd_kwargs)
    return finish_output(res.results), res


def kernel(**inputs):
    out, _ = run(inputs)
    return out
